# revision 2
# baseline (speedup 1.0000x reference)
"""Trainium2 Bass kernel for CovarianceSimilarity — trace-trick rewrite.

Math: scores[n,w] = sum_k conv_w[k]*lrelu(F[n,w,k]) + cb with
F[n,w,k] = qn_k^T cov_w qn_k.  cov_w is PSD, so F >= lambda_min*||qn||^2
(measured F in [0.345, 1.75] on this input distribution) and the
LeakyReLU is exactly the identity.  Therefore

  scores[n,w] = <cov_w, M_n>_F + cb,   M_n = Qn diag(conv_w) Qn^T,

which collapses the per-(query,way) matmul loop into one M_n per query
(5x fewer PE MACs) plus 5 cheap Frobenius dots.  On this stack wall time
is ~163ms fixed invocation overhead + ~4us per instruction + ~10ps/byte,
so the kernel minimizes instruction count:

  - Cov phase: each core computes partial uncentered moments A_w (upper
    block-triangle, bank-aligned psum layout shared with M) over its 1/8
    of the Shot*d samples, plus sample row-sums; ONE merged f32 AllReduce
    carries both (s rows packed into extra payload columns).  Rank-1
    mean correction + 2x off-diagonal-block weighting produce V2 tiles so
    that <cov, M> = sum(V2 o M_upper) exactly (diag blocks kept full).
  - Per query (10/core): one 1.3MB bf16 DMA (k-major qt layout), squares
    + grouped tensor_reduce + ones-matmul partition sum for the channel
    norms, PE outer-product broadcast of 1/norm, ONE fused broadcast
    multiply each for Qn^T and conv_w-scaled Qn^T (f32r), 48 PSUM-
    accumulated matmuls for M_n (upper triangle, 6 bank-contained runs
    x 8 k-blocks), and 5 scalar_tensor_tensor ops (M o V2_w, free-axis
    accumulate) producing the per-way scores.
  - 1/(N-1+eps) is folded into conv_w on the host; gaps in the
    bank-aligned layout are zeroed in V2 so full-span dots are exact.
  - gpsimd only issues the collective; all broadcasts/reductions that
    cross partitions go through the PE (ones matmuls / outer products).
"""

import numpy as np
from contextlib import ExitStack

import concourse.bass as bass
import concourse.tile as tile
from concourse import bacc, mybir
from concourse.bass_utils import run_bass_kernel_spmd

# ---- problem constants (hardcoded per spec) ----
NQ, C, D = 75, 640, 1024
WAY, SHOT = 5, 5
NTOT = SHOT * D            # 5120 samples per way
NCORES = 8
QPC = 10                   # queries per core (NQ padded to 80)
NCHUNK = NTOT // NCORES    # 640 sample columns per core
EPS = 1e-8
CB = C // 128              # 5 channel blocks (m)
KB = D // 128              # 8 k-blocks
NB = NCHUNK // 128         # 5 sample chunks per core
SCALE = 1.0 / (NTOT - 1 + EPS)
F32 = mybir.dt.float32
F32R = mybir.dt.float32r
BF16 = mybir.dt.bfloat16

# Upper-block-triangle layouts.  PSUM: every accumulation group gets a
# 2KB bank of its own (start_tensor_calc resets the whole bank, so two
# open groups must never share one).  Block-row m holds columns
# b in [m*128, 640).
PMOFF = [0, 1024, 1536, 2048, 2560]
WID = [640, 512, 384, 256, 128]
PSUMW = 2688               # 6 banks
MRUNS = {0: [(0, 512), (512, 640)], 1: [(1024, 1536)], 2: [(1536, 1920)],
         3: [(2048, 2304)], 4: [(2560, 2688)]}
# dense SBUF/DRAM layout for drained A and M (and V2)
DOFF = [0, 640, 1152, 1536, 1792]
DTOT = 1920
ACOLS = WAY * DTOT         # 9600 A columns in the AllReduce payload
SOFF = ACOLS               # s rows live at [0:1, SOFF + w*C : ...]
PAYC = ACOLS + WAY * C     # 12800 total payload columns

_CACHE = {}
RG = [list(range(NCORES))]
DEBUG = False


def _build():
    nc = bacc.Bacc("TRN2", target_bir_lowering=False, debug=False,
                   num_devices=NCORES)
    qt_ap = nc.dram_tensor("qt", [QPC, 128, KB * C], BF16,
                           kind="ExternalInput").ap()
    st_ap = nc.dram_tensor("st", [128, WAY * NB * C], F32R,
                           kind="ExternalInput").ap()
    cw_ap = nc.dram_tensor("cw", [128, KB], F32, kind="ExternalInput").ap()
    cb_ap = nc.dram_tensor("cb", [1, 1], F32, kind="ExternalInput").ap()
    out_ap = nc.dram_tensor("out", [1, QPC * WAY], F32,
                            kind="ExternalOutput").ap()
    dbg = {}
    if DEBUG:
        dbg["v2"] = nc.dram_tensor("d_v2", [128, DTOT], F32,
                                   kind="ExternalOutput").ap()
        dbg["ssq"] = nc.dram_tensor("d_ssq", [128, C], F32,
                                    kind="ExternalOutput").ap()
        dbg["nub"] = nc.dram_tensor("d_nub", [128, C], BF16,
                                    kind="ExternalOutput").ap()
        dbg["qtn"] = nc.dram_tensor("d_qtn", [128, KB * C], F32,
                                    kind="ExternalOutput").ap()
        dbg["qtcw"] = nc.dram_tensor("d_qtcw", [128, KB * C], F32,
                                     kind="ExternalOutput").ap()
        dbg["racc"] = nc.dram_tensor("d_racc", [128, QPC * WAY], F32,
                                     kind="ExternalOutput").ap()
        dbg["ain"] = nc.dram_tensor("d_ain", [128, PAYC], F32,
                                    kind="ExternalOutput").ap()
        dbg["aout"] = nc.dram_tensor("d_aout", [128, PAYC], F32,
                                     kind="ExternalOutput").ap()

    with tile.TileContext(nc) as tc:
        with ExitStack() as ctx:
            _body(nc, tc, ctx, qt_ap, st_ap, cw_ap, cb_ap, out_ap, dbg)
    nc.compile()
    return nc


def _body(nc, tc, ctx, qt_ap, st_ap, cw_ap, cb_ap, out_ap, dbg={}):
    # ---------------- persistent pools ----------------
    vpool = ctx.enter_context(tc.tile_pool(name="v2", bufs=1))
    const = ctx.enter_context(tc.tile_pool(name="const", bufs=1))
    rpool = ctx.enter_context(tc.tile_pool(name="r", bufs=1))

    ones32 = const.tile([128, 1], F32, tag="ones32")
    nc.vector.memset(ones32[:], 1.0)
    onesr = const.tile([128, 1], F32R, tag="onesr")
    nc.vector.tensor_copy(onesr[:], ones32[:])
    orow32 = const.tile([1, 128], F32, tag="orow32")
    nc.vector.memset(orow32[:], 1.0)
    orowr = const.tile([1, 128], F32R, tag="orowr")
    nc.vector.tensor_copy(orowr[:], orow32[:])
    cw_sb = const.tile([128, KB], F32, tag="cw")
    nc.sync.dma_start(cw_sb[:], cw_ap[:])
    cb_sb = const.tile([1, 1], F32, tag="cb")
    nc.sync.dma_start(cb_sb[:], cb_ap[:])

    v2 = [vpool.tile([128, DTOT], BF16, tag=f"v2_{w}", name=f"v2_{w}")
          for w in range(WAY)]
    racc = rpool.tile([128, QPC * WAY], F32, tag="racc")

    # ---------------- phase A: partial moments + merged AllReduce ----
    with tc.tile_pool(name="covdram", bufs=1, space="DRAM") as cov_dram, \
         ExitStack() as cov_ctx:
        a_in = cov_dram.tile([128, PAYC], BF16)
        a_out = cov_dram.tile([128, PAYC], BF16, addr_space="Shared")

        stp = cov_ctx.enter_context(tc.tile_pool(name="straw", bufs=1))
        st_w = []
        for w in range(WAY):
            t = stp.tile([128, NB * C], F32R, tag=f"st{w}", name=f"st{w}")
            nc.sync.dma_start(t[:], st_ap[:, w * NB * C:(w + 1) * NB * C])
            st_w.append(t)

        with tc.tile_pool(name="apsum", bufs=1, space="PSUM") as apsum, \
             tc.tile_pool(name="spsum", bufs=1, space="PSUM") as spsum, \
             tc.tile_pool(name="sbdrain", bufs=2) as sbdrain:
            for w in range(WAY):
                def st_slc(nb, lo, hi):
                    return st_w[w][:, nb * C + lo:nb * C + hi]
                # s_w row: ones^T @ st, accumulated over sample chunks
                s_ps = spsum.tile([1, C], F32, tag="sps")
                for nb in range(NB):
                    for lo, hi in ((0, 512), (512, 640)):
                        nc.tensor.matmul(s_ps[:, lo:hi], onesr[:],
                                         st_slc(nb, lo, hi),
                                         start=(nb == 0), stop=(nb == NB - 1),
                                         skip_group_check=True)
                s_row = sbdrain.tile([1, C], BF16, tag="srow")
                nc.vector.tensor_copy(s_row[:], s_ps[:])
                nc.sync.dma_start(a_in[0:1, SOFF + w * C:SOFF + (w + 1) * C],
                                  s_row[:])
                # A upper block-triangle; every open group in its own bank
                a_ps = apsum.tile([128, PSUMW], F32, tag="aps")
                for nb in range(NB):
                    for m in range(CB):
                        for plo, phi in MRUNS[m]:
                            blo = m * 128 + (plo - PMOFF[m])
                            nc.tensor.matmul(
                                a_ps[:, plo:phi],
                                st_slc(nb, m * 128, (m + 1) * 128),
                                st_slc(nb, blo, blo + (phi - plo)),
                                start=(nb == 0), stop=(nb == NB - 1),
                                skip_group_check=True)
                a_sb = sbdrain.tile([128, DTOT], BF16, tag="adrain")
                for m in range(CB):
                    nc.vector.tensor_copy(
                        a_sb[:, DOFF[m]:DOFF[m] + WID[m]],
                        a_ps[:, PMOFF[m]:PMOFF[m] + WID[m]])
                nc.sync.dma_start(a_in[:, w * DTOT:(w + 1) * DTOT], a_sb[:])

        nc.gpsimd.collective_compute(
            "AllReduce", mybir.AluOpType.add,
            replica_groups=RG,
            ins=[a_in.opt()], outs=[a_out.opt()],
        )
        if dbg:
            nc.sync.dma_start(dbg["ain"][:], a_in[:])
            nc.sync.dma_start(dbg["aout"][:], a_out[:])

        # ---- V2 prep:  V2 = weight * (A - s s^T / NTOT) ----
        sprp = cov_ctx.enter_context(tc.tile_pool(name="sprep", bufs=2))
        alod = cov_ctx.enter_context(tc.tile_pool(name="aload", bufs=2))
        with tc.tile_pool(name="opsum", bufs=1, space="PSUM") as opsum:
            for w in range(WAY):
                srow = sprp.tile([1, C], BF16, tag="sraw")
                nc.sync.dma_start(srow[:],
                                  a_out[0:1, SOFF + w * C:SOFF + (w + 1) * C])
                s1 = sprp.tile([1, C], F32R, tag="s1")
                nc.vector.tensor_scalar_mul(s1[:], srow[:],
                                            float(np.sqrt(1.0 / NTOT)))
                s2 = sprp.tile([1, C], F32R, tag="s2")
                nc.vector.tensor_scalar_mul(s2[:], srow[:],
                                            float(np.sqrt(2.0 / NTOT)))
                a_all = alod.tile([128, DTOT], BF16, tag="aload")
                nc.sync.dma_start(a_all[:], a_out[:, w * DTOT:(w + 1) * DTOT])
                vt = v2[w]
                for m in range(CB):
                    # fresh (tag-reused) 2-bank tile per m: diag group in
                    # bank 0, off-diagonal group in bank 1
                    o_ps = opsum.tile([128, 1024], F32, tag="ops")
                    nc.tensor.matmul(o_ps[:, 0:128],
                                     s1[0:1, m * 128:(m + 1) * 128],
                                     s1[0:1, m * 128:(m + 1) * 128],
                                     start=True, stop=True,
                                     skip_group_check=True)
                    offw = WID[m] - 128
                    if offw:
                        blo = (m + 1) * 128
                        nc.tensor.matmul(o_ps[:, 512:512 + offw],
                                         s2[0:1, m * 128:(m + 1) * 128],
                                         s2[0:1, blo:blo + offw],
                                         start=True, stop=True,
                                         skip_group_check=True)
                    dlo = DOFF[m]
                    nc.vector.scalar_tensor_tensor(
                        vt[:, dlo:dlo + 128], a_all[:, dlo:dlo + 128], 1.0,
                        o_ps[:, 0:128],
                        op0=mybir.AluOpType.mult,
                        op1=mybir.AluOpType.subtract)
                    if offw:
                        nc.vector.scalar_tensor_tensor(
                            vt[:, dlo + 128:dlo + WID[m]],
                            a_all[:, dlo + 128:dlo + WID[m]], 2.0,
                            o_ps[:, 512:512 + offw],
                            op0=mybir.AluOpType.mult,
                            op1=mybir.AluOpType.subtract)

    # ---------------- phase B: queries ----------------
    qtp = ctx.enter_context(tc.tile_pool(name="qt", bufs=2))
    sqp = ctx.enter_context(tc.tile_pool(name="sq", bufs=1))
    nrm = ctx.enter_context(tc.tile_pool(name="nrm", bufs=2))
    qnp = ctx.enter_context(tc.tile_pool(name="qn", bufs=2))
    qwp = ctx.enter_context(tc.tile_pool(name="qw", bufs=2))
    scr = ctx.enter_context(tc.tile_pool(name="scr", bufs=2))

    with tc.tile_pool(name="mpsum", bufs=1, space="PSUM") as mpsum, \
         tc.tile_pool(name="npsum", bufs=1, space="PSUM") as npsum:
        for q in range(QPC):
            qt = qtp.tile([128, KB * C], BF16, tag="qt")
            nc.sync.dma_start(qt[:], qt_ap[q])
            qt3 = qt[:].rearrange("p (kb c) -> p kb c", kb=KB, c=C)
            # channel norms: sum_k qt^2 (grouped reduce + partition sum)
            sq = sqp.tile([128, KB * C], BF16, tag="sq")
            nc.vector.tensor_tensor(sq[:], qt[:], qt[:],
                                    op=mybir.AluOpType.mult)
            ssq = nrm.tile([128, C], F32R, tag="ssq")
            with nc.allow_low_precision(reason="f32r is 4-byte"):
                nc.vector.tensor_reduce(
                    ssq[:], sq[:].rearrange("p (kb c) -> p c kb", kb=KB, c=C),
                    axis=mybir.AxisListType.X, op=mybir.AluOpType.add)
            s_ps = npsum.tile([128, C], F32, tag="np")
            for lo, hi in ((0, 512), (512, 640)):
                nc.tensor.matmul(s_ps[0:1, lo:hi], onesr[:],
                                 ssq[:, lo:hi],
                                 start=True, stop=True,
                                 skip_group_check=True)
            nrow = nrm.tile([1, C], F32, tag="nrow")
            nc.scalar.activation(nrow[:], s_ps[0:1, :],
                                 mybir.ActivationFunctionType.Sqrt)
            nc.vector.tensor_scalar_add(nrow[:], nrow[:], EPS)
            invr = nrm.tile([1, C], F32R, tag="invr")
            with nc.allow_low_precision(reason="f32r is 4-byte"):
                nc.vector.reciprocal(invr[:], nrow[:])
            # broadcast 1/norm to all partitions via PE outer product
            nu_b = nrm.tile([128, C], BF16, tag="nub")
            for lo, hi in ((0, 512), (512, 640)):
                b_ps = npsum.tile([128, C], F32, tag="np")
                nc.tensor.matmul(b_ps[:, 0:hi - lo], orowr[:],
                                 invr[0:1, lo:hi], start=True, stop=True,
                                 skip_group_check=True)
                nc.vector.tensor_copy(nu_b[:, lo:hi], b_ps[:, 0:hi - lo])
            # qtn = qt * nu_b  (f32r), qtcw = qtn * cw*SCALE (f32r)
            qtn = qnp.tile([128, KB * C], F32R, tag="qtn")
            nc.vector.tensor_tensor(
                qtn[:].rearrange("p (kb c) -> p kb c", kb=KB, c=C), qt3,
                nu_b[:].unsqueeze(1).broadcast_to([128, KB, C]),
                op=mybir.AluOpType.mult)
            qtcw = qwp.tile([128, KB * C], F32R, tag="qtcw")
            nc.vector.tensor_tensor(
                qtcw[:].rearrange("p (kb c) -> p kb c", kb=KB, c=C),
                qtn[:].bitcast(F32).rearrange("p (kb c) -> p kb c",
                                              kb=KB, c=C),
                cw_sb[:].unsqueeze(2).broadcast_to([128, KB, C]),
                op=mybir.AluOpType.mult)
            # M_n upper block-triangle, accumulated over k-blocks
            mps = mpsum.tile([128, PSUMW], F32, tag="mps")
            for kb in range(KB):
                for m in range(CB):
                    for plo, phi in MRUNS[m]:
                        blo = m * 128 + (plo - PMOFF[m])
                        nc.tensor.matmul(
                            mps[:, plo:phi],
                            qtcw[:, kb * C + m * 128:kb * C + (m + 1) * 128],
                            qtn[:, kb * C + blo:kb * C + blo + (phi - plo)],
                            start=(kb == 0), stop=(kb == KB - 1),
                            skip_group_check=True)
            m_sb = scr.tile([128, DTOT], BF16, tag="msb")
            for m in range(CB):
                nc.vector.tensor_copy(m_sb[:, DOFF[m]:DOFF[m] + WID[m]],
                                      mps[:, PMOFF[m]:PMOFF[m] + WID[m]])
            # per-way scores: sum(V2_w o M)
            for w in range(WAY):
                sc = scr.tile([128, DTOT], BF16, tag="sct")
                nc.vector.scalar_tensor_tensor(
                    sc[:], m_sb[:], 1.0, v2[w][:],
                    op0=mybir.AluOpType.mult, op1=mybir.AluOpType.mult,
                    accum_out=racc[:, q * WAY + w:q * WAY + w + 1])
            if dbg and q == 0:
                nc.sync.dma_start(dbg["ssq"][:], ssq[:].bitcast(F32))
                nc.sync.dma_start(dbg["nub"][:], nu_b[:])
                nc.sync.dma_start(dbg["qtn"][:], qtn[:].bitcast(F32))
                nc.sync.dma_start(dbg["qtcw"][:], qtcw[:].bitcast(F32))

    # ---------------- final: scores ----------------
    with tc.tile_pool(name="fin", bufs=1, space="PSUM") as fin, \
         tc.tile_pool(name="osb", bufs=1) as osb:
        racc_r = rpool.tile([128, QPC * WAY], F32R, tag="raccr")
        nc.vector.tensor_copy(racc_r[:], racc[:])
        sc_ps = fin.tile([1, QPC * WAY], F32, tag="scps")
        nc.tensor.matmul(sc_ps[:], onesr[:], racc_r[:],
                         start=True, stop=True)
        sc_sb = osb.tile([1, QPC * WAY], F32, tag="scsb")
        nc.vector.tensor_scalar_add(sc_sb[:], sc_ps[:], cb_sb[:])
        nc.sync.dma_start(out_ap[:], sc_sb[:])
        if dbg:
            nc.sync.dma_start(dbg["v2"][:], v2[0][:])
            nc.sync.dma_start(dbg["racc"][:], racc[:])


def _get_nc():
    if "nc" not in _CACHE:
        _CACHE["nc"] = _build()
    return _CACHE["nc"]


def _to_bf16(x):
    import ml_dtypes
    return np.asarray(x, dtype=np.float32).astype(ml_dtypes.bfloat16)


def _host_prep(query, support, conv_w, conv_b):
    q = np.ascontiguousarray(query.reshape(NQ, C, D), dtype=np.float32)
    pad = NCORES * QPC - NQ
    qpad = np.concatenate([q, np.broadcast_to(q[0:1], (pad, C, D))], axis=0)
    # qt: [80][128][KB*C], partition p col (kb*C + c) holds Q[n, c, kb*128+p]
    qt_full = np.ascontiguousarray(
        qpad.transpose(0, 2, 1).reshape(NCORES * QPC, KB, 128, C)
        .transpose(0, 2, 1, 3).reshape(NCORES * QPC, 128, KB * C))
    # st: per core [128][WAY*NB*C]; col (w,nb,c): sample nb*128+p of way w
    st_full = np.ascontiguousarray(
        support.transpose(0, 2, 1, 3, 4).reshape(WAY, C, NTOT),
        dtype=np.float32)                     # [WAY, C, NTOT]
    st_t = st_full.transpose(0, 2, 1)         # [WAY, NTOT, C]
    cw = np.ascontiguousarray(
        (conv_w * SCALE).reshape(KB, 128).T, dtype=np.float32)
    cb = np.asarray(conv_b, dtype=np.float32).reshape(1, 1)
    in_maps = []
    for c in range(NCORES):
        qt = _to_bf16(qt_full[c * QPC:(c + 1) * QPC])
        stc = st_t[:, c * NCHUNK:(c + 1) * NCHUNK, :]     # [WAY, 640, C]
        st = np.ascontiguousarray(
            stc.reshape(WAY, NB, 128, C).transpose(2, 0, 1, 3)
            .reshape(128, WAY * NB * C), dtype=np.float32)
        in_maps.append({"qt": qt, "st": st, "cw": cw, "cb": cb})
    return in_maps


def kernel(query, support, conv_w, conv_b):
    in_maps = _host_prep(np.asarray(query), np.asarray(support),
                         np.asarray(conv_w), np.asarray(conv_b))
    nc = _get_nc()
    res = run_bass_kernel_spmd(nc, in_maps, core_ids=list(range(NCORES)))
    scores = np.concatenate(
        [res.results[c]["out"].reshape(QPC, WAY) for c in range(NCORES)],
        axis=0)[:NQ]
    return np.ascontiguousarray(scores, dtype=np.float32)


if __name__ == "__main__":
    import reference
    inputs = reference.setup_inputs()
    exp = np.asarray(reference.reference(**inputs))
    got = kernel(**{k: np.asarray(v) for k, v in inputs.items()})
    rel = np.abs(got - exp).max() / np.abs(exp).max()
    print(f"Relative error: {rel:.3e}")


# revision 4
# speedup vs baseline: 1.0074x; 1.0074x over previous
"""Trainium2 Bass kernel for CovarianceSimilarity — trace-trick rewrite.

Math: scores[n,w] = sum_k conv_w[k]*lrelu(F[n,w,k]) + cb with
F[n,w,k] = qn_k^T cov_w qn_k.  cov_w is PSD, so F >= lambda_min*||qn||^2
(measured F in [0.345, 1.75] on this input distribution) and the
LeakyReLU is exactly the identity.  Therefore

  scores[n,w] = <cov_w, M_n>_F + cb,   M_n = Qn diag(conv_w) Qn^T,

which collapses the per-(query,way) matmul loop into one M_n per query
(5x fewer PE MACs) plus 5 cheap Frobenius dots.  On this stack wall time
is ~163ms fixed invocation overhead + ~4us per instruction + ~10ps/byte,
so the kernel minimizes instruction count:

  - Cov phase: each core computes partial uncentered moments A_w (upper
    block-triangle; every open PSUM accumulation group owns its 2KB bank
    exclusively) over its 1/8 of the Shot*d samples, plus sample
    row-sums; ONE merged bf16 AllReduce carries both (s rows packed into
    extra payload columns).  Rank-1 mean correction + 2x off-diagonal-
    block weighting produce V2 tiles so that <cov, M> =
    sum(V2 o M_upper) exactly (diag blocks kept full).
  - Per query (10/core): one 1.3MB bf16 DMA (k-major qt layout), squares
    + grouped tensor_reduce + ones-matmul partition sum for the channel
    norms, PE outer-product broadcast of 1/norm, ONE fused broadcast
    multiply each for Qn^T and conv_w-scaled Qn^T (f32r), 48 PSUM-
    accumulated matmuls for M_n (upper triangle, 6 bank-contained runs
    x 8 k-blocks), and 5 scalar_tensor_tensor ops (M o V2_w, free-axis
    accumulate) producing the per-way scores.
  - 1/(N-1+eps) is folded into conv_w on the host; A/M drain from the
    bank-aligned psum layout into a dense bf16 SBUF layout that V2
    matches, so the score dots are single full-span ops.
  - gpsimd only issues the collective; all broadcasts/reductions that
    cross partitions go through the PE (ones matmuls / outer products).
"""

import numpy as np
from contextlib import ExitStack

import concourse.bass as bass
import concourse.tile as tile
from concourse import bacc, mybir
from concourse.bass_utils import run_bass_kernel_spmd

# ---- problem constants (hardcoded per spec) ----
NQ, C, D = 75, 640, 1024
WAY, SHOT = 5, 5
NTOT = SHOT * D            # 5120 samples per way
NCORES = 8
QPC = 10                   # queries per core (NQ padded to 80)
NCHUNK = NTOT // NCORES    # 640 sample columns per core
EPS = 1e-8
CB = C // 128              # 5 channel blocks (m)
KB = D // 128              # 8 k-blocks
NB = NCHUNK // 128         # 5 sample chunks per core
SCALE = 1.0 / (NTOT - 1 + EPS)
F32 = mybir.dt.float32
F32R = mybir.dt.float32r
BF16 = mybir.dt.bfloat16

# Upper-block-triangle layouts.  PSUM: every accumulation group gets a
# 2KB bank of its own (start_tensor_calc resets the whole bank, so two
# open groups must never share one).  Block-row m holds columns
# b in [m*128, 640).
PMOFF = [0, 1024, 1536, 2048, 2560]
WID = [640, 512, 384, 256, 128]
PSUMW = 2688               # 6 banks
MRUNS = {0: [(0, 512), (512, 640)], 1: [(1024, 1536)], 2: [(1536, 1920)],
         3: [(2048, 2304)], 4: [(2560, 2688)]}
# dense SBUF/DRAM layout for drained A and M (and V2)
DOFF = [0, 640, 1152, 1536, 1792]
DTOT = 1920
ACOLS = WAY * DTOT         # 9600 A columns in the AllReduce payload
SOFF = ACOLS               # s rows live at [0:1, SOFF + w*C : ...]
PAYC = ACOLS + WAY * C     # 12800 total payload columns

_CACHE = {}
RG = [list(range(NCORES))]
DEBUG = False


def _build():
    nc = bacc.Bacc("TRN2", target_bir_lowering=False, debug=False,
                   num_devices=NCORES)
    qt_ap = nc.dram_tensor("qt", [QPC, 128, KB * C], BF16,
                           kind="ExternalInput").ap()
    st_ap = nc.dram_tensor("st", [128, WAY * NB * C], F32R,
                           kind="ExternalInput").ap()
    cw_ap = nc.dram_tensor("cw", [128, KB], F32, kind="ExternalInput").ap()
    cb_ap = nc.dram_tensor("cb", [1, 1], F32, kind="ExternalInput").ap()
    out_ap = nc.dram_tensor("out", [1, QPC * WAY], F32,
                            kind="ExternalOutput").ap()
    dbg = {}
    if DEBUG:
        dbg["v2"] = nc.dram_tensor("d_v2", [128, DTOT], F32,
                                   kind="ExternalOutput").ap()
        dbg["ssq"] = nc.dram_tensor("d_ssq", [128, C], F32,
                                    kind="ExternalOutput").ap()
        dbg["nub"] = nc.dram_tensor("d_nub", [128, C], BF16,
                                    kind="ExternalOutput").ap()
        dbg["qtn"] = nc.dram_tensor("d_qtn", [128, KB * C], F32,
                                    kind="ExternalOutput").ap()
        dbg["qtcw"] = nc.dram_tensor("d_qtcw", [128, KB * C], F32,
                                     kind="ExternalOutput").ap()
        dbg["racc"] = nc.dram_tensor("d_racc", [128, QPC * WAY], F32,
                                     kind="ExternalOutput").ap()
        dbg["ain"] = nc.dram_tensor("d_ain", [128, PAYC], F32,
                                    kind="ExternalOutput").ap()
        dbg["aout"] = nc.dram_tensor("d_aout", [128, PAYC], F32,
                                     kind="ExternalOutput").ap()

    with tile.TileContext(nc) as tc:
        with ExitStack() as ctx:
            _body(nc, tc, ctx, qt_ap, st_ap, cw_ap, cb_ap, out_ap, dbg)
    nc.compile()
    return nc


def _body(nc, tc, ctx, qt_ap, st_ap, cw_ap, cb_ap, out_ap, dbg={}):
    # ---------------- persistent pools ----------------
    vpool = ctx.enter_context(tc.tile_pool(name="v2", bufs=1))
    const = ctx.enter_context(tc.tile_pool(name="const", bufs=1))
    rpool = ctx.enter_context(tc.tile_pool(name="r", bufs=1))

    ones32 = const.tile([128, 1], F32, tag="ones32")
    nc.vector.memset(ones32[:], 1.0)
    onesr = const.tile([128, 1], F32R, tag="onesr")
    nc.vector.tensor_copy(onesr[:], ones32[:])
    orow32 = const.tile([1, 128], F32, tag="orow32")
    nc.vector.memset(orow32[:], 1.0)
    orowr = const.tile([1, 128], F32R, tag="orowr")
    nc.vector.tensor_copy(orowr[:], orow32[:])
    cw_sb = const.tile([128, KB], F32, tag="cw")
    nc.sync.dma_start(cw_sb[:], cw_ap[:])
    cb_sb = const.tile([1, 1], F32, tag="cb")
    nc.sync.dma_start(cb_sb[:], cb_ap[:])

    v2 = [vpool.tile([128, DTOT], BF16, tag=f"v2_{w}", name=f"v2_{w}")
          for w in range(WAY)]
    racc = rpool.tile([128, QPC * WAY], F32, tag="racc")

    # ---------------- phase A: partial moments + merged AllReduce ----
    with tc.tile_pool(name="covdram", bufs=1, space="DRAM") as cov_dram, \
         ExitStack() as cov_ctx:
        a_in = cov_dram.tile([128, PAYC], BF16)
        a_out = cov_dram.tile([128, PAYC], BF16, addr_space="Shared")

        stp = cov_ctx.enter_context(tc.tile_pool(name="straw", bufs=1))
        st_w = []
        for w in range(WAY):
            t = stp.tile([128, NB * C], F32R, tag=f"st{w}", name=f"st{w}")
            nc.sync.dma_start(t[:], st_ap[:, w * NB * C:(w + 1) * NB * C])
            st_w.append(t)

        with tc.tile_pool(name="apsum", bufs=1, space="PSUM") as apsum, \
             tc.tile_pool(name="spsum", bufs=1, space="PSUM") as spsum, \
             tc.tile_pool(name="sbdrain", bufs=2) as sbdrain:
            for w in range(WAY):
                def st_slc(nb, lo, hi):
                    return st_w[w][:, nb * C + lo:nb * C + hi]
                # s_w row: ones^T @ st, accumulated over sample chunks
                s_ps = spsum.tile([1, C], F32, tag="sps")
                for nb in range(NB):
                    for lo, hi in ((0, 512), (512, 640)):
                        nc.tensor.matmul(s_ps[:, lo:hi], onesr[:],
                                         st_slc(nb, lo, hi),
                                         start=(nb == 0), stop=(nb == NB - 1),
                                         skip_group_check=True)
                s_row = sbdrain.tile([1, C], BF16, tag="srow")
                nc.vector.tensor_copy(s_row[:], s_ps[:])
                nc.sync.dma_start(a_in[0:1, SOFF + w * C:SOFF + (w + 1) * C],
                                  s_row[:])
                # A upper block-triangle; every open group in its own bank
                a_ps = apsum.tile([128, PSUMW], F32, tag="aps")
                for nb in range(NB):
                    for m in range(CB):
                        for plo, phi in MRUNS[m]:
                            blo = m * 128 + (plo - PMOFF[m])
                            nc.tensor.matmul(
                                a_ps[:, plo:phi],
                                st_slc(nb, m * 128, (m + 1) * 128),
                                st_slc(nb, blo, blo + (phi - plo)),
                                start=(nb == 0), stop=(nb == NB - 1),
                                skip_group_check=True)
                a_sb = sbdrain.tile([128, DTOT], BF16, tag="adrain")
                for m in range(CB):
                    nc.vector.tensor_copy(
                        a_sb[:, DOFF[m]:DOFF[m] + WID[m]],
                        a_ps[:, PMOFF[m]:PMOFF[m] + WID[m]])
                nc.sync.dma_start(a_in[:, w * DTOT:(w + 1) * DTOT], a_sb[:])

        nc.gpsimd.collective_compute(
            "AllReduce", mybir.AluOpType.add,
            replica_groups=RG,
            ins=[a_in.opt()], outs=[a_out.opt()],
        )
        if dbg:
            nc.sync.dma_start(dbg["ain"][:], a_in[:])
            nc.sync.dma_start(dbg["aout"][:], a_out[:])

        # ---- V2 prep:  V2 = weight * (A - s s^T / NTOT) ----
        sprp = cov_ctx.enter_context(tc.tile_pool(name="sprep", bufs=2))
        alod = cov_ctx.enter_context(tc.tile_pool(name="aload", bufs=2))
        with tc.tile_pool(name="opsum", bufs=1, space="PSUM") as opsum:
            for w in range(WAY):
                srow = sprp.tile([1, C], BF16, tag="sraw")
                nc.sync.dma_start(srow[:],
                                  a_out[0:1, SOFF + w * C:SOFF + (w + 1) * C])
                s1 = sprp.tile([1, C], F32R, tag="s1")
                nc.vector.tensor_scalar_mul(s1[:], srow[:],
                                            float(np.sqrt(1.0 / NTOT)))
                s2 = sprp.tile([1, C], F32R, tag="s2")
                nc.vector.tensor_scalar_mul(s2[:], srow[:],
                                            float(np.sqrt(2.0 / NTOT)))
                a_all = alod.tile([128, DTOT], BF16, tag="aload")
                nc.sync.dma_start(a_all[:], a_out[:, w * DTOT:(w + 1) * DTOT])
                vt = v2[w]
                for m in range(CB):
                    # fresh (tag-reused) 2-bank tile per m: diag group in
                    # bank 0, off-diagonal group in bank 1
                    o_ps = opsum.tile([128, 1024], F32, tag="ops")
                    nc.tensor.matmul(o_ps[:, 0:128],
                                     s1[0:1, m * 128:(m + 1) * 128],
                                     s1[0:1, m * 128:(m + 1) * 128],
                                     start=True, stop=True,
                                     skip_group_check=True)
                    offw = WID[m] - 128
                    if offw:
                        blo = (m + 1) * 128
                        nc.tensor.matmul(o_ps[:, 512:512 + offw],
                                         s2[0:1, m * 128:(m + 1) * 128],
                                         s2[0:1, blo:blo + offw],
                                         start=True, stop=True,
                                         skip_group_check=True)
                    dlo = DOFF[m]
                    nc.vector.scalar_tensor_tensor(
                        vt[:, dlo:dlo + 128], a_all[:, dlo:dlo + 128], 1.0,
                        o_ps[:, 0:128],
                        op0=mybir.AluOpType.mult,
                        op1=mybir.AluOpType.subtract)
                    if offw:
                        nc.vector.scalar_tensor_tensor(
                            vt[:, dlo + 128:dlo + WID[m]],
                            a_all[:, dlo + 128:dlo + WID[m]], 2.0,
                            o_ps[:, 512:512 + offw],
                            op0=mybir.AluOpType.mult,
                            op1=mybir.AluOpType.subtract)

    # ---------------- phase B: queries ----------------
    qtp = ctx.enter_context(tc.tile_pool(name="qt", bufs=2))
    sqp = ctx.enter_context(tc.tile_pool(name="sq", bufs=1))
    nrm = ctx.enter_context(tc.tile_pool(name="nrm", bufs=2))
    qnp = ctx.enter_context(tc.tile_pool(name="qn", bufs=2))
    qwp = ctx.enter_context(tc.tile_pool(name="qw", bufs=2))
    scr = ctx.enter_context(tc.tile_pool(name="scr", bufs=2))

    with tc.tile_pool(name="mpsum", bufs=1, space="PSUM") as mpsum, \
         tc.tile_pool(name="npsum", bufs=1, space="PSUM") as npsum:
        for q in range(QPC):
            qt = qtp.tile([128, KB * C], BF16, tag="qt")
            nc.sync.dma_start(qt[:], qt_ap[q])
            qt3 = qt[:].rearrange("p (kb c) -> p kb c", kb=KB, c=C)
            # channel norms: sum_k qt^2 (grouped reduce + partition sum)
            sq = sqp.tile([128, KB * C], BF16, tag="sq")
            nc.vector.tensor_tensor(sq[:], qt[:], qt[:],
                                    op=mybir.AluOpType.mult)
            ssq = nrm.tile([128, C], F32R, tag="ssq")
            with nc.allow_low_precision(reason="f32r is 4-byte"):
                nc.vector.tensor_reduce(
                    ssq[:], sq[:].rearrange("p (kb c) -> p c kb", kb=KB, c=C),
                    axis=mybir.AxisListType.X, op=mybir.AluOpType.add)
            s_ps = npsum.tile([128, C], F32, tag="np")
            for lo, hi in ((0, 512), (512, 640)):
                nc.tensor.matmul(s_ps[0:1, lo:hi], onesr[:],
                                 ssq[:, lo:hi],
                                 start=True, stop=True,
                                 skip_group_check=True)
            nrow = nrm.tile([1, C], F32, tag="nrow")
            nc.scalar.activation(nrow[:], s_ps[0:1, :],
                                 mybir.ActivationFunctionType.Sqrt)
            nc.vector.tensor_scalar_add(nrow[:], nrow[:], EPS)
            invr = nrm.tile([1, C], F32R, tag="invr")
            with nc.allow_low_precision(reason="f32r is 4-byte"):
                nc.vector.reciprocal(invr[:], nrow[:])
            # broadcast 1/norm to all partitions via PE outer product
            nu_b = nrm.tile([128, C], BF16, tag="nub")
            for lo, hi in ((0, 512), (512, 640)):
                b_ps = npsum.tile([128, C], F32, tag="np")
                nc.tensor.matmul(b_ps[:, 0:hi - lo], orowr[:],
                                 invr[0:1, lo:hi], start=True, stop=True,
                                 skip_group_check=True)
                nc.vector.tensor_copy(nu_b[:, lo:hi], b_ps[:, 0:hi - lo])
            # qtn = qt * nu_b  (f32r), qtcw = qtn * cw*SCALE (f32r)
            qtn = qnp.tile([128, KB * C], F32R, tag="qtn")
            nc.vector.tensor_tensor(
                qtn[:].rearrange("p (kb c) -> p kb c", kb=KB, c=C), qt3,
                nu_b[:].unsqueeze(1).broadcast_to([128, KB, C]),
                op=mybir.AluOpType.mult)
            qtcw = qwp.tile([128, KB * C], F32R, tag="qtcw")
            nc.vector.tensor_tensor(
                qtcw[:].rearrange("p (kb c) -> p kb c", kb=KB, c=C),
                qtn[:].bitcast(F32).rearrange("p (kb c) -> p kb c",
                                              kb=KB, c=C),
                cw_sb[:].unsqueeze(2).broadcast_to([128, KB, C]),
                op=mybir.AluOpType.mult)
            # M_n upper block-triangle, accumulated over k-blocks
            mps = mpsum.tile([128, PSUMW], F32, tag="mps")
            for kb in range(KB):
                for m in range(CB):
                    for plo, phi in MRUNS[m]:
                        blo = m * 128 + (plo - PMOFF[m])
                        nc.tensor.matmul(
                            mps[:, plo:phi],
                            qtcw[:, kb * C + m * 128:kb * C + (m + 1) * 128],
                            qtn[:, kb * C + blo:kb * C + blo + (phi - plo)],
                            start=(kb == 0), stop=(kb == KB - 1),
                            skip_group_check=True)
            m_sb = scr.tile([128, DTOT], BF16, tag="msb")
            for m in range(CB):
                nc.vector.tensor_copy(m_sb[:, DOFF[m]:DOFF[m] + WID[m]],
                                      mps[:, PMOFF[m]:PMOFF[m] + WID[m]])
            # per-way scores: sum(V2_w o M)
            for w in range(WAY):
                sc = scr.tile([128, DTOT], BF16, tag="sct")
                nc.vector.scalar_tensor_tensor(
                    sc[:], m_sb[:], 1.0, v2[w][:],
                    op0=mybir.AluOpType.mult, op1=mybir.AluOpType.mult,
                    accum_out=racc[:, q * WAY + w:q * WAY + w + 1])
            if dbg and q == 0:
                nc.sync.dma_start(dbg["ssq"][:], ssq[:].bitcast(F32))
                nc.sync.dma_start(dbg["nub"][:], nu_b[:])
                nc.sync.dma_start(dbg["qtn"][:], qtn[:].bitcast(F32))
                nc.sync.dma_start(dbg["qtcw"][:], qtcw[:].bitcast(F32))

    # ---------------- final: scores ----------------
    with tc.tile_pool(name="fin", bufs=1, space="PSUM") as fin, \
         tc.tile_pool(name="osb", bufs=1) as osb:
        racc_r = rpool.tile([128, QPC * WAY], F32R, tag="raccr")
        nc.vector.tensor_copy(racc_r[:], racc[:])
        sc_ps = fin.tile([1, QPC * WAY], F32, tag="scps")
        nc.tensor.matmul(sc_ps[:], onesr[:], racc_r[:],
                         start=True, stop=True)
        sc_sb = osb.tile([1, QPC * WAY], F32, tag="scsb")
        nc.vector.tensor_scalar_add(sc_sb[:], sc_ps[:], cb_sb[:])
        nc.sync.dma_start(out_ap[:], sc_sb[:])
        if dbg:
            nc.sync.dma_start(dbg["v2"][:], v2[0][:])
            nc.sync.dma_start(dbg["racc"][:], racc[:])


def _get_nc():
    if "nc" not in _CACHE:
        _CACHE["nc"] = _build()
    return _CACHE["nc"]


def _to_bf16(x):
    import ml_dtypes
    return np.asarray(x, dtype=np.float32).astype(ml_dtypes.bfloat16)


def _host_prep(query, support, conv_w, conv_b):
    q = np.ascontiguousarray(query.reshape(NQ, C, D), dtype=np.float32)
    pad = NCORES * QPC - NQ
    qpad = np.concatenate([q, np.broadcast_to(q[0:1], (pad, C, D))], axis=0)
    # qt: [80][128][KB*C], partition p col (kb*C + c) holds Q[n, c, kb*128+p]
    qt_full = np.ascontiguousarray(
        qpad.transpose(0, 2, 1).reshape(NCORES * QPC, KB, 128, C)
        .transpose(0, 2, 1, 3).reshape(NCORES * QPC, 128, KB * C))
    # st: per core [128][WAY*NB*C]; col (w,nb,c): sample nb*128+p of way w
    st_full = np.ascontiguousarray(
        support.transpose(0, 2, 1, 3, 4).reshape(WAY, C, NTOT),
        dtype=np.float32)                     # [WAY, C, NTOT]
    st_t = st_full.transpose(0, 2, 1)         # [WAY, NTOT, C]
    cw = np.ascontiguousarray(
        (conv_w * SCALE).reshape(KB, 128).T, dtype=np.float32)
    cb = np.asarray(conv_b, dtype=np.float32).reshape(1, 1)
    in_maps = []
    for c in range(NCORES):
        qt = _to_bf16(qt_full[c * QPC:(c + 1) * QPC])
        stc = st_t[:, c * NCHUNK:(c + 1) * NCHUNK, :]     # [WAY, 640, C]
        st = np.ascontiguousarray(
            stc.reshape(WAY, NB, 128, C).transpose(2, 0, 1, 3)
            .reshape(128, WAY * NB * C), dtype=np.float32)
        in_maps.append({"qt": qt, "st": st, "cw": cw, "cb": cb})
    return in_maps


def kernel(query, support, conv_w, conv_b):
    in_maps = _host_prep(np.asarray(query), np.asarray(support),
                         np.asarray(conv_w), np.asarray(conv_b))
    nc = _get_nc()
    res = run_bass_kernel_spmd(nc, in_maps, core_ids=list(range(NCORES)))
    scores = np.concatenate(
        [res.results[c]["out"].reshape(QPC, WAY) for c in range(NCORES)],
        axis=0)[:NQ]
    return np.ascontiguousarray(scores, dtype=np.float32)


if __name__ == "__main__":
    import reference
    inputs = reference.setup_inputs()
    exp = np.asarray(reference.reference(**inputs))
    got = kernel(**{k: np.asarray(v) for k, v in inputs.items()})
    rel = np.abs(got - exp).max() / np.abs(exp).max()
    print(f"Relative error: {rel:.3e}")
